# revision 7
# baseline (speedup 1.0000x reference)
"""DGCNN encoder Trainium2 kernel (batch-parallel over 8 NeuronCores).

Per core, one sample x (3, 2048). EdgeConv collapses algebraically:
with f = cat(nbr-ctr, ctr), conv+BN+ReLU+max over k becomes
  x_out[o,n] = relu( max_{m in knn(n)} P[o,m] + Q[o,n] )
  P = (s*W_nbr) x,  Q = (s*(W_ctr-W_nbr)) x + (s*(b-mu)+beta),  s >= 0.
KNN scores score[n,m] = 2<x_n,x_m> - |x_m|^2 (row-constant term dropped).
Top-20 per row: DVE max8/max_index/match_replace rounds. Neighbor max of P:
per-rank indirect DMA row-gathers of P^T (DRAM) + in-place DVE max fold.
"""
import numpy as np

import concourse.bacc as bacc
import concourse.bass as bass
import concourse.mybir as mybir
from concourse.tile import TileContext
from concourse.bass_utils import run_bass_kernel_spmd

F32 = mybir.dt.float32
U32 = mybir.dt.uint32
AX = mybir.AluOpType
AF = mybir.ActivationFunctionType

N = 2048
K = 20
NT = N // 128
EPS = 1e-5
NEG = -1e30

LAYERS = [(3, 64), (64, 128), (128, 256)]

_cache = {}


def _fold_host(inputs):
    out = {}
    for li, (C, O) in enumerate(LAYERS, start=1):
        w = inputs[f'w{li}']; b = inputs[f'b{li}']; g = inputs[f'g{li}']
        be = inputs[f'be{li}']; m = inputs[f'm{li}']; v = inputs[f'v{li}']
        s = g / np.sqrt(v + EPS)
        A = (s[:, None] * w[:, :C]).astype(np.float32)
        B = (s[:, None] * (w[:, C:] - w[:, :C])).astype(np.float32)
        c = (s * (b - m) + be).astype(np.float32)
        out[f'AT{li}'] = np.ascontiguousarray(A.T)
        out[f'BT{li}'] = np.ascontiguousarray(B.T)
        out[f'cb{li}'] = np.ascontiguousarray(c[None, :])
    so = inputs['go'] / np.sqrt(inputs['vo'] + EPS)
    Ao = (so[:, None] * inputs['wo']).astype(np.float32)
    co = (so * (inputs['bo'] - inputs['mo']) + inputs['beo']).astype(np.float32)
    AoT = np.ascontiguousarray(Ao.T)
    out['AoT1'] = np.ascontiguousarray(AoT[0:64])
    out['AoT2'] = np.ascontiguousarray(AoT[64:192])
    out['AoT3a'] = np.ascontiguousarray(AoT[192:320])
    out['AoT3b'] = np.ascontiguousarray(AoT[320:448])
    out['co'] = np.ascontiguousarray(co.reshape(4, 128).T)
    return out


class _Builder:
    def __init__(self):
        self.nc = bacc.Bacc(None, target_bir_lowering=False, debug=False)
        self.d = {}

    def inp(self, name, shape):
        self.d[name] = self.nc.dram_tensor(name, shape, F32, kind="ExternalInput")

    def edge_layer(self, x, li, C, O):
        nc, pp, wp = self.nc, self.pp, self.wp
        mmps, auxps = self.mmps, self.auxps
        ones = self.ones
        ATs = pp.tile([C, O], F32, name=f"ATs{li}", tag=f"ATs{li}")
        BTs = pp.tile([C, O], F32, name=f"BTs{li}", tag=f"BTs{li}")
        cbs = pp.tile([1, O], F32, name=f"cbs{li}", tag=f"cbs{li}")
        nc.sync.dma_start(ATs[:], self.d[f'AT{li}'][:])
        nc.sync.dma_start(BTs[:], self.d[f'BT{li}'][:])
        nc.sync.dma_start(cbs[:], self.d[f'cb{li}'][:])

        PTd = self.dpool.tile([N, O], F32, name=f"PTd{li}", tag=f"PTd{li}")
        for t in range(NT):
            pt_ps = auxps.tile([128, O], F32, name=f"ptps{li}_{t}", tag="aux_ps",
                               space="PSUM")
            nc.tensor.matmul(out=pt_ps[:], lhsT=x[:, t*128:(t+1)*128], rhs=ATs[:],
                             start=True, stop=True)
            pt_sb = wp.tile([128, O], F32, name=f"ptsb{li}_{t}", tag="pt_sb")
            nc.scalar.copy(out=pt_sb[:], in_=pt_ps[:])
            nc.sync.dma_start(PTd[t*128:(t+1)*128, :], pt_sb[:])

        aug = wp.tile([C, N], F32, name=f"aug{li}", tag="aug")
        negxx = wp.tile([1, N], F32, name=f"negxx{li}", tag="negxx")
        sq = wp.tile([C, N], F32, name=f"sq{li}", tag="sq")
        nc.vector.tensor_tensor(out=sq[:], in0=x[:], in1=x[:], op=AX.mult)
        nc.scalar.activation(out=aug[:], in_=x[:], func=AF.Copy, scale=2.0)
        for ch in range(4):
            xx_ps = auxps.tile([1, 512], F32, name=f"xxps{li}_{ch}", tag="aux_ps",
                               space="PSUM")
            nc.tensor.matmul(out=xx_ps[:], lhsT=ones[0:C, 0:1],
                             rhs=sq[:, ch*512:(ch+1)*512], start=True, stop=True)
            nc.scalar.activation(out=negxx[0:1, ch*512:(ch+1)*512], in_=xx_ps[:],
                                 func=AF.Copy, scale=-1.0)

        nob = max(1, O // 128)
        x_next = [self.pp.tile([min(128, O), N], F32, name=f"xn{li}_{i}", tag=f"xn{li}_{i}")
                  for i in range(nob)]

        for t in range(NT):
            tsl = slice(t * 128, (t + 1) * 128)
            scoreS = wp.tile([128, N], F32, name=f"sc{li}_{t}", tag="scoreS")
            for ch in range(4):
                sc_ps = mmps.tile([128, 512], F32, name=f"scps{li}_{t}_{ch}", tag="mm_ps",
                                  space="PSUM")
                csl = slice(ch * 512, (ch + 1) * 512)
                nc.tensor.matmul(out=sc_ps[:], lhsT=x[:, tsl], rhs=aug[:, csl],
                                 start=True, stop=False)
                nc.tensor.matmul(out=sc_ps[:], lhsT=ones[0:1, 0:128], rhs=negxx[0:1, csl],
                                 start=False, stop=True)
                nc.scalar.copy(out=scoreS[:, csl], in_=sc_ps[:])

            mx = wp.tile([128, 24], F32, name=f"mx{li}_{t}", tag="mx")
            ix = wp.tile([128, 24], U32, name=f"ix{li}_{t}", tag="ix")
            for r in range(3):
                rsl = slice(r * 8, (r + 1) * 8)
                nc.vector.max(out=mx[:, rsl], in_=scoreS[:])
                nc.vector.max_index(out=ix[:, rsl], in_max=mx[:, rsl], in_values=scoreS[:])
                if r < 2:
                    nc.vector.match_replace(out=scoreS[:], in_to_replace=mx[:, rsl],
                                            in_values=scoreS[:], imm_value=NEG)

            qt_ps = auxps.tile([128, O], F32, name=f"qtps{li}_{t}", tag="aux_ps",
                               space="PSUM")
            nc.tensor.matmul(out=qt_ps[:], lhsT=x[:, tsl], rhs=BTs[:], start=True,
                             stop=False)
            nc.tensor.matmul(out=qt_ps[:], lhsT=ones[0:1, 0:128], rhs=cbs[:],
                             start=False, stop=True)

            g = wp.tile([128, K * O], F32, name=f"g{li}_{t}", tag="gath")
            for j in range(K):
                nc.gpsimd.indirect_dma_start(
                    out=g[:, j*O:(j+1)*O], out_offset=None, in_=PTd[:],
                    in_offset=bass.IndirectOffsetOnAxis(ap=ix[:, j:j+1], axis=0))

            nc.vector.tensor_tensor(out=g[:, :10*O], in0=g[:, :10*O],
                                    in1=g[:, 10*O:20*O], op=AX.max)
            nc.vector.tensor_tensor(out=g[:, :5*O], in0=g[:, :5*O], in1=g[:, 5*O:10*O],
                                    op=AX.max)
            nc.vector.tensor_tensor(out=g[:, :2*O], in0=g[:, :2*O], in1=g[:, 2*O:4*O],
                                    op=AX.max)
            nc.vector.tensor_tensor(out=g[:, :O], in0=g[:, :O], in1=g[:, O:2*O], op=AX.max)
            nc.vector.tensor_tensor(out=g[:, :O], in0=g[:, :O], in1=g[:, 4*O:5*O], op=AX.max)
            nc.vector.tensor_tensor(out=g[:, :O], in0=g[:, :O], in1=qt_ps[:], op=AX.add)
            nc.vector.tensor_scalar_max(g[:, :O], g[:, :O], 0.0)

            for ob in range(nob):
                obs = slice(ob * 128, min((ob + 1) * 128, O))
                w = obs.stop - obs.start
                tp_ps = auxps.tile([128, 128], F32, name=f"tp{li}_{t}_{ob}", tag="aux_ps",
                                   space="PSUM")
                nc.tensor.transpose(out=tp_ps[0:w, :], in_=g[:, obs], identity=self.ident[:])
                nc.scalar.copy(out=x_next[ob][0:w, tsl], in_=tp_ps[0:w, :])
        return x_next

    def build(self):
        nc = self.nc
        self.inp('x', [3, N])
        for li, (C, O) in enumerate(LAYERS, start=1):
            self.inp(f'AT{li}', [C, O]); self.inp(f'BT{li}', [C, O])
            self.inp(f'cb{li}', [1, O])
        self.inp('AoT1', [64, 512]); self.inp('AoT2', [128, 512])
        self.inp('AoT3a', [128, 512]); self.inp('AoT3b', [128, 512])
        self.inp('co', [128, 4]); self.inp('identity', [128, 128])
        out_d = nc.dram_tensor('out', [512], F32, kind="ExternalOutput")

        with TileContext(nc) as tc:
            with (
                tc.tile_pool(name="pp", bufs=1) as pp,
                tc.tile_pool(name="wp", bufs=2) as wp,
                tc.tile_pool(name="mmps", bufs=3, space="PSUM") as mmps,
                tc.tile_pool(name="auxps", bufs=3, space="PSUM") as auxps,
                tc.tile_pool(name="dram", bufs=1, space="DRAM") as dpool,
            ):
                self.pp, self.wp = pp, wp
                self.mmps, self.auxps, self.dpool = mmps, auxps, dpool
                ones = pp.tile([128, 128], F32, name="ones", tag="ones")
                nc.vector.memset(ones[:], 1.0)
                self.ones = ones
                ident = pp.tile([128, 128], F32, name="identS", tag="identS")
                nc.sync.dma_start(ident[:], self.d['identity'][:])
                self.ident = ident
                x0 = pp.tile([3, N], F32, name="x0", tag="x0")
                nc.sync.dma_start(x0[:], self.d['x'][:])

                x1 = self.edge_layer(x0, 1, 3, 64)[0]
                x2 = self.edge_layer(x1, 2, 64, 128)[0]
                x3a, x3b = self.edge_layer(x2, 3, 128, 256)

                specs = [('AoT1', x1, 64), ('AoT2', x2, 128),
                         ('AoT3a', x3a, 128), ('AoT3b', x3b, 128)]
                lhs_s = []
                for i, (nm, _, kk) in enumerate(specs):
                    ls = pp.tile([kk, 512], F32, name=f"Ao{i}", tag=f"Ao{i}")
                    nc.sync.dma_start(ls[:], self.d[nm][:])
                    lhs_s.append(ls)
                cos = pp.tile([128, 4], F32, name="cos", tag="cos")
                nc.sync.dma_start(cos[:], self.d['co'][:])

                for mc in range(4):
                    msl = slice(mc * 128, (mc + 1) * 128)
                    acc = wp.tile([128, 4], F32, name=f"acc{mc}", tag="acc")
                    red = wp.tile([128, 1], F32, name=f"red{mc}", tag="red")
                    for nchk in range(4):
                        nsl = slice(nchk * 512, (nchk + 1) * 512)
                        y_ps = mmps.tile([128, 512], F32, name=f"y{mc}_{nchk}",
                                         tag="mm_ps", space="PSUM")
                        for ki, (_, xs, kk) in enumerate(specs):
                            nc.tensor.matmul(out=y_ps[:], lhsT=lhs_s[ki][:, msl],
                                             rhs=xs[0:kk, nsl],
                                             start=(ki == 0), stop=(ki == 3))
                        y_sb = wp.tile([128, 512], F32, name=f"ysb{mc}_{nchk}", tag="y_sb")
                        nc.scalar.activation(out=y_sb[:], in_=y_ps[:], func=AF.Relu,
                                             bias=cos[:, mc:mc+1], scale=1.0)
                        nc.vector.tensor_reduce(out=acc[:, nchk:nchk+1], in_=y_sb[:],
                                                axis=mybir.AxisListType.X, op=AX.max)
                    nc.vector.tensor_reduce(out=red[:], in_=acc[:],
                                            axis=mybir.AxisListType.X, op=AX.max)
                    nc.sync.dma_start(out_d[msl], red[:])
        nc.compile()
        return nc


def build_kernel():
    return _Builder().build()


def kernel(**inputs):
    if 'nc' not in _cache:
        _cache['nc'] = build_kernel()
    nc = _cache['nc']
    folded = _fold_host(inputs)
    base = {**folded, 'identity': np.eye(128, dtype=np.float32)}
    xs = np.asarray(inputs['x'], dtype=np.float32)
    in_maps = [{**base, 'x': np.ascontiguousarray(xs[b])} for b in range(8)]
    res = run_bass_kernel_spmd(nc, in_maps, core_ids=list(range(8)))
    return np.stack([res.results[b]['out'] for b in range(8)]).astype(np.float32)
